# revision 7
# baseline (speedup 1.0000x reference)
"""VQ codebook kernel for Trainium2 (8 NeuronCores, data-parallel over batch).

Problem: x (32, 256, 32, 32) f32, lookup_table (1, 1, 2048, 256) f32.
Per token (b, n) with n in [0, 1024): q = argmin_k ||x[b,:,n] - codes[k]||^2,
x_e = codes[q] laid back out as (b, d, h, w), plus a "faithful" codebook loss
that compares the (b, hw, d) buffer raw-reshaped as (b, d, h, w) against x_e.

Device mapping (per core, 4 batch items):
  - cross[n, k] via PE fp32 matmul (K-split 128+128), accumulated in PSUM;
    the -0.5*||c_k||^2 bias row (host-precomputed, replicated over the 128
    partitions) is added by DVE while moving scores PSUM -> SBUF.
  - argmax over k: DVE max (fp32 top-8, exact) -> DVE max_index (first
    occurrence of the max, matching jnp.argmin first-tie semantics).
  - gather codes[q] via GPSIMD indirect DMA (row gather).
  - x_e output layout via PE transposes, DMA straight out of PSUM.
  - loss = (sum x^2 + sum x_e^2 - 2*crossterm) / N, where crossterm uses the
    identity  vec(x_t) . vec(x_e^T) = sum_j <x[b][:, j::4], x_e_flat tile>,
    so no transposes are needed; per-partition partials reduced on host.
"""

import numpy as np

B, D, H, W = 32, 256, 32, 32
HW = H * W            # 1024
K = 2048
NCORES = 8
BPC = B // NCORES     # 4 batch items per core
P = 128
NCHUNK = HW // P      # 8 token chunks per batch item
CHUNKS = BPC * NCHUNK  # 32 per core

_CACHE = {}


def _build_program():
    import concourse.bacc as bacc
    import concourse.mybir as mybir
    from concourse import bass
    from concourse.bass import IndirectOffsetOnAxis
    from concourse.masks import make_identity
    from concourse.tile import TileContext

    f32 = mybir.dt.float32
    bf16 = mybir.dt.bfloat16
    u32 = mybir.dt.uint32
    AF = mybir.ActivationFunctionType
    OP = mybir.AluOpType

    nc = bacc.Bacc("TRN2", target_bir_lowering=False, debug=False)

    x_d = nc.declare_dram_parameter("x_sh", [BPC, D, HW], f32, isOutput=False)
    lut_d = nc.declare_dram_parameter("lut", [K, D], f32, isOutput=False)
    lutT_d = nc.declare_dram_parameter("lutT", [D, K], f32, isOutput=False)
    bias_d = nc.declare_dram_parameter("bias_rep", [P, K], f32, isOutput=False)
    xe_d = nc.declare_dram_parameter("xe_sh", [BPC, D, HW], f32, isOutput=True)
    q_d = nc.declare_dram_parameter("q_sh", [BPC, HW], u32, isOutput=True)
    acc_d = nc.declare_dram_parameter("acc", [P, 72], f32, isOutput=True)

    with TileContext(nc) as tc:
        with (
            tc.tile_pool(name="const", bufs=1) as cpool,
            tc.tile_pool(name="xin", bufs=4) as xpool,
            tc.tile_pool(name="sb16", bufs=2) as sbpool,
            tc.tile_pool(name="small", bufs=3) as mpool,
            tc.tile_pool(name="xef", bufs=3) as gpool,
            tc.tile_pool(name="scr", bufs=3) as scrpool,
            tc.tile_pool(name="spsum", bufs=3, space="PSUM") as spool,
            tc.tile_pool(name="tpsum", bufs=2, space="PSUM") as tpool,
        ):
            lutT_lo = cpool.tile([P, K], f32, tag="lutT_lo")
            lutT_hi = cpool.tile([P, K], f32, tag="lutT_hi")
            nc.sync.dma_start(out=lutT_lo[:], in_=lutT_d[0:P, :])
            nc.sync.dma_start(out=lutT_hi[:], in_=lutT_d[P:D, :])
            bias_sb = cpool.tile([P, K], f32, tag="bias")
            nc.sync.dma_start(out=bias_sb[:], in_=bias_d[:, :])
            ident = cpool.tile([P, P], f32, tag="ident")
            make_identity(nc, ident[:])
            acc_cross = cpool.tile([P, CHUNKS], f32, tag="acc_cross")
            acc_sxe = cpool.tile([P, CHUNKS], f32, tag="acc_sxe")
            acc_sxx = cpool.tile([P, 2 * BPC], f32, tag="acc_sxx")

            for b in range(BPC):
                x_lo = xpool.tile([P, HW], f32, tag="x")
                x_hi = xpool.tile([P, HW], f32, tag="x")
                nc.sync.dma_start(out=x_lo[:], in_=x_d[b, 0:P, :])
                nc.sync.dma_start(out=x_hi[:], in_=x_d[b, P:D, :])

                # sum(x^2) partials on ACT (square + accumulate)
                scx = scrpool.tile([P, HW], f32, tag="scr")
                nc.scalar.activation(
                    out=scx[:], in_=x_lo[:], func=AF.Square,
                    accum_out=acc_sxx[:, 2 * b : 2 * b + 1],
                )
                scx2 = scrpool.tile([P, HW], f32, tag="scr")
                nc.scalar.activation(
                    out=scx2[:], in_=x_hi[:], func=AF.Square,
                    accum_out=acc_sxx[:, 2 * b + 1 : 2 * b + 2],
                )

                for m in range(NCHUNK):
                    ci = b * NCHUNK + m
                    msl = slice(m * P, (m + 1) * P)
                    s_lo = spool.tile([P, K // 2], f32, tag="s")
                    s_hi = spool.tile([P, K // 2], f32, tag="s")
                    # scores: s[n, k] = x.T @ lutT + (-0.5*csq) row
                    for half, s_t in ((0, s_lo), (1, s_hi)):
                        for nb in range(2):
                            col = slice(nb * 512, (nb + 1) * 512)
                            kcol = slice(half * 1024 + nb * 512,
                                         half * 1024 + (nb + 1) * 512)
                            nc.tensor.matmul(
                                out=s_t[:, col], lhsT=x_lo[:, msl],
                                rhs=lutT_lo[:, kcol], start=True, stop=False,
                            )
                            nc.tensor.matmul(
                                out=s_t[:, col], lhsT=x_hi[:, msl],
                                rhs=lutT_hi[:, kcol], start=False, stop=True,
                            )

                    # add the -0.5*||c||^2 bias row (replicated across
                    # partitions) while moving scores PSUM -> SBUF fp32
                    sb = sbpool.tile([P, K], f32, tag="sb")
                    nc.vector.tensor_tensor(
                        out=sb[:, 0:1024], in0=s_lo[:],
                        in1=bias_sb[:, 0:1024], op=OP.add,
                    )
                    nc.vector.tensor_tensor(
                        out=sb[:, 1024:2048], in0=s_hi[:],
                        in1=bias_sb[:, 1024:2048], op=OP.add,
                    )
                    # exact fp32 argmax: top-8 then first-occurrence index
                    mm = mpool.tile([P, 8], f32, tag="mm")
                    nc.vector.max(out=mm[:], in_=sb[:])
                    idx = mpool.tile([P, 8], u32, tag="idx")
                    nc.vector.max_index(
                        out=idx[:], in_max=mm[:], in_values=sb[:]
                    )
                    nc.sync.dma_start(out=q_d[b, msl], in_=idx[:, 0:1])

                    # gather codes[q] -> (128 tokens, 256)
                    xef = gpool.tile([P, D], f32, tag="xef")
                    nc.gpsimd.indirect_dma_start(
                        out=xef[:], out_offset=None, in_=lut_d[:, :],
                        in_offset=IndirectOffsetOnAxis(ap=idx[:, 0:1], axis=0),
                    )

                    # loss partials: crossterm (DVE) + sum(x_e^2) (ACT)
                    xsrc = x_lo if m % 2 == 0 else x_hi
                    j = m // 2
                    strided = xsrc.rearrange("p (n four) -> p four n", four=4)[:, j, :]
                    scr = scrpool.tile([P, HW], f32, tag="scr")
                    # tensor_tensor_reduce wedges the device on this HW path;
                    # use separate mult + reduce
                    nc.vector.tensor_tensor(
                        out=scr[:, 0:D], in0=strided, in1=xef[:], op=OP.mult
                    )
                    nc.vector.tensor_reduce(
                        out=acc_cross[:, ci : ci + 1], in_=scr[:, 0:D],
                        axis=mybir.AxisListType.X, op=OP.add,
                    )
                    scr2 = scrpool.tile([P, HW], f32, tag="scr")
                    nc.scalar.activation(
                        out=scr2[:, 0:D], in_=xef[:], func=AF.Square,
                        accum_out=acc_sxe[:, ci : ci + 1],
                    )

                    # x_e output: PE-transpose 128x128 blocks into one PSUM
                    # bank, bounce through SBUF (DMA can't read PSUM)
                    tp = tpool.tile([P, D], f32, tag="tp")
                    for eh in range(2):
                        esl = slice(eh * P, (eh + 1) * P)
                        nc.tensor.transpose(
                            out=tp[:, esl], in_=xef[:, esl], identity=ident[:]
                        )
                    xeT = gpool.tile([P, D], f32, tag="xeT")
                    nc.scalar.copy(out=xeT[:], in_=tp[:])
                    dout = xe_d[b].rearrange("(two e) hw -> e two hw", two=2)
                    nc.sync.dma_start(out=dout[:, :, msl], in_=xeT[:])

            nc.sync.dma_start(out=acc_d[:, 0:32], in_=acc_cross[:])
            nc.sync.dma_start(out=acc_d[:, 32:64], in_=acc_sxe[:])
            nc.sync.dma_start(out=acc_d[:, 64:72], in_=acc_sxx[:])

    nc.compile()
    return nc


def _get_program():
    if "nc" not in _CACHE:
        _CACHE["nc"] = _build_program()
    return _CACHE["nc"]


def _make_in_maps(x, lut):
    lutT = np.ascontiguousarray(lut.T)
    csq = (lut.astype(np.float64) ** 2).sum(axis=1)
    bias_rep = np.broadcast_to((-0.5 * csq).astype(np.float32).reshape(1, K), (128, K)).copy()
    in_maps = []
    for i in range(NCORES):
        in_maps.append({
            "x_sh": np.ascontiguousarray(
                x[i * BPC : (i + 1) * BPC].reshape(BPC, D, HW)),
            "lut": lut,
            "lutT": lutT,
            "bias_rep": bias_rep,
        })
    return in_maps


def kernel(x: np.ndarray, lookup_table: np.ndarray):
    x = np.asarray(x, dtype=np.float32)
    lut = np.ascontiguousarray(
        np.asarray(lookup_table, dtype=np.float32).reshape(K, D))

    from concourse.bass_utils import run_bass_kernel_spmd

    nc = _get_program()
    in_maps = _make_in_maps(x, lut)
    res = run_bass_kernel_spmd(nc, in_maps, list(range(NCORES))).results

    x_e = np.concatenate(
        [res[i]["xe_sh"].reshape(BPC, D, H, W) for i in range(NCORES)], axis=0)
    q_x = np.concatenate(
        [res[i]["q_sh"].astype(np.int32).reshape(BPC, H, W)
         for i in range(NCORES)], axis=0)

    tot = 0.0
    for i in range(NCORES):
        acc = res[i]["acc"].astype(np.float64)
        cross = acc[:, 0:32].sum()
        sxe = acc[:, 32:64].sum()
        sxx = acc[:, 64:72].sum()
        tot += sxx + sxe - 2.0 * cross
    loss = np.float32(tot / (B * D * HW))

    return x_e, q_x, loss


# revision 8
# speedup vs baseline: 1.5121x; 1.5121x over previous
"""VQ codebook kernel for Trainium2 (8 NeuronCores, data-parallel over batch).

Problem: x (32, 256, 32, 32) f32, lookup_table (1, 1, 2048, 256) f32.
Per token (b, n) with n in [0, 1024): q = argmin_k ||x[b,:,n] - codes[k]||^2,
x_e = codes[q] laid back out as (b, d, h, w), plus a "faithful" codebook loss
that compares the (b, hw, d) buffer raw-reshaped as (b, d, h, w) against x_e.

Device mapping (per core, 4 batch items):
  - cross[n, k] via PE fp32 matmul (K-split 128+128), accumulated in PSUM;
    the -0.5*||c_k||^2 bias row (host-precomputed, replicated over the 128
    partitions) is added by DVE while moving scores PSUM -> SBUF.
  - argmax over k: DVE max (fp32 top-8, exact) -> DVE max_index (first
    occurrence of the max, matching jnp.argmin first-tie semantics).
  - gather codes[q] via GPSIMD indirect DMA (row gather).
  - x_e output layout via PE transposes, DMA straight out of PSUM.
  - loss = (sum x^2 + sum x_e^2 - 2*crossterm) / N, where crossterm uses the
    identity  vec(x_t) . vec(x_e^T) = sum_j <x[b][:, j::4], x_e_flat tile>,
    so no transposes are needed; per-partition partials reduced on host.
"""

import numpy as np

B, D, H, W = 32, 256, 32, 32
HW = H * W            # 1024
K = 2048
NCORES = 8
BPC = B // NCORES     # 4 batch items per core
P = 128
NCHUNK = HW // P      # 8 token chunks per batch item
CHUNKS = BPC * NCHUNK  # 32 per core

_CACHE = {}


def _build_program():
    import concourse.bacc as bacc
    import concourse.mybir as mybir
    from concourse import bass
    from concourse.bass import IndirectOffsetOnAxis
    from concourse.masks import make_identity
    from concourse.tile import TileContext

    f32 = mybir.dt.float32
    bf16 = mybir.dt.bfloat16
    u32 = mybir.dt.uint32
    AF = mybir.ActivationFunctionType
    OP = mybir.AluOpType

    nc = bacc.Bacc("TRN2", target_bir_lowering=False, debug=False)

    x_d = nc.declare_dram_parameter("x_sh", [BPC, D, HW], f32, isOutput=False)
    lut_d = nc.declare_dram_parameter("lut", [K, D], f32, isOutput=False)
    lutT_d = nc.declare_dram_parameter("lutT", [D, K], f32, isOutput=False)
    bias_d = nc.declare_dram_parameter("bias_rep", [P, K], f32, isOutput=False)
    xe_d = nc.declare_dram_parameter("xe_sh", [BPC, D, HW], f32, isOutput=True)
    q_d = nc.declare_dram_parameter("q_sh", [BPC, HW], u32, isOutput=True)
    acc_d = nc.declare_dram_parameter("acc", [P, 72], f32, isOutput=True)

    with TileContext(nc) as tc:
        with (
            tc.tile_pool(name="const", bufs=1) as cpool,
            tc.tile_pool(name="xin", bufs=4) as xpool,
            tc.tile_pool(name="sb16", bufs=2) as sbpool,
            tc.tile_pool(name="small", bufs=3) as mpool,
            tc.tile_pool(name="xef", bufs=3) as gpool,
            tc.tile_pool(name="scr", bufs=3) as scrpool,
            tc.tile_pool(name="spsum", bufs=3, space="PSUM") as spool,
            tc.tile_pool(name="tpsum", bufs=2, space="PSUM") as tpool,
        ):
            lutT_lo = cpool.tile([P, K], f32, tag="lutT_lo")
            lutT_hi = cpool.tile([P, K], f32, tag="lutT_hi")
            nc.sync.dma_start(out=lutT_lo[:], in_=lutT_d[0:P, :])
            nc.sync.dma_start(out=lutT_hi[:], in_=lutT_d[P:D, :])
            bias_sb = cpool.tile([P, K], f32, tag="bias")
            nc.sync.dma_start(out=bias_sb[:], in_=bias_d[:, :])
            ident = cpool.tile([P, P], f32, tag="ident")
            make_identity(nc, ident[:])
            acc_cross = cpool.tile([P, CHUNKS], f32, tag="acc_cross")
            acc_sxe = cpool.tile([P, CHUNKS], f32, tag="acc_sxe")
            acc_sxx = cpool.tile([P, 2 * BPC], f32, tag="acc_sxx")

            for b in range(BPC):
                x_lo = xpool.tile([P, HW], f32, tag="x")
                x_hi = xpool.tile([P, HW], f32, tag="x")
                nc.sync.dma_start(out=x_lo[:], in_=x_d[b, 0:P, :])
                nc.sync.dma_start(out=x_hi[:], in_=x_d[b, P:D, :])

                # sum(x^2) partials on ACT (square + accumulate)
                scx = scrpool.tile([P, HW], f32, tag="scr")
                nc.scalar.activation(
                    out=scx[:], in_=x_lo[:], func=AF.Square,
                    accum_out=acc_sxx[:, 2 * b : 2 * b + 1],
                )
                scx2 = scrpool.tile([P, HW], f32, tag="scr")
                nc.scalar.activation(
                    out=scx2[:], in_=x_hi[:], func=AF.Square,
                    accum_out=acc_sxx[:, 2 * b + 1 : 2 * b + 2],
                )

                for m in range(NCHUNK):
                    ci = b * NCHUNK + m
                    msl = slice(m * P, (m + 1) * P)
                    s_lo = spool.tile([P, K // 2], f32, tag="s")
                    s_hi = spool.tile([P, K // 2], f32, tag="s")
                    # scores: s[n, k] = x.T @ lutT + (-0.5*csq) row
                    for half, s_t in ((0, s_lo), (1, s_hi)):
                        for nb in range(2):
                            col = slice(nb * 512, (nb + 1) * 512)
                            kcol = slice(half * 1024 + nb * 512,
                                         half * 1024 + (nb + 1) * 512)
                            nc.tensor.matmul(
                                out=s_t[:, col], lhsT=x_lo[:, msl],
                                rhs=lutT_lo[:, kcol], start=True, stop=False,
                            )
                            nc.tensor.matmul(
                                out=s_t[:, col], lhsT=x_hi[:, msl],
                                rhs=lutT_hi[:, kcol], start=False, stop=True,
                            )

                    # add the -0.5*||c||^2 bias row (replicated across
                    # partitions) while moving scores PSUM -> SBUF fp32
                    sb = sbpool.tile([P, K], f32, tag="sb")
                    nc.vector.tensor_tensor(
                        out=sb[:, 0:1024], in0=s_lo[:],
                        in1=bias_sb[:, 0:1024], op=OP.add,
                    )
                    nc.vector.tensor_tensor(
                        out=sb[:, 1024:2048], in0=s_hi[:],
                        in1=bias_sb[:, 1024:2048], op=OP.add,
                    )
                    # exact fp32 argmax: top-8 then first-occurrence index
                    mm = mpool.tile([P, 8], f32, tag="mm")
                    nc.vector.max(out=mm[:], in_=sb[:])
                    idx = mpool.tile([P, 8], u32, tag="idx")
                    nc.vector.max_index(
                        out=idx[:], in_max=mm[:], in_values=sb[:]
                    )
                    nc.sync.dma_start(out=q_d[b, msl], in_=idx[:, 0:1])

                    # gather codes[q] -> (128 tokens, 256)
                    xef = gpool.tile([P, D], f32, tag="xef")
                    nc.gpsimd.indirect_dma_start(
                        out=xef[:], out_offset=None, in_=lut_d[:, :],
                        in_offset=IndirectOffsetOnAxis(ap=idx[:, 0:1], axis=0),
                    )

                    # loss partials: crossterm (DVE) + sum(x_e^2) (ACT)
                    xsrc = x_lo if m % 2 == 0 else x_hi
                    j = m // 2
                    strided = xsrc.rearrange("p (n four) -> p four n", four=4)[:, j, :]
                    scr = scrpool.tile([P, HW], f32, tag="scr")
                    # tensor_tensor_reduce wedges the device on this HW path;
                    # use separate mult + reduce
                    nc.vector.tensor_tensor(
                        out=scr[:, 0:D], in0=strided, in1=xef[:], op=OP.mult
                    )
                    nc.vector.tensor_reduce(
                        out=acc_cross[:, ci : ci + 1], in_=scr[:, 0:D],
                        axis=mybir.AxisListType.X, op=OP.add,
                    )
                    scr2 = scrpool.tile([P, HW], f32, tag="scr")
                    nc.scalar.activation(
                        out=scr2[:, 0:D], in_=xef[:], func=AF.Square,
                        accum_out=acc_sxe[:, ci : ci + 1],
                    )

                    # x_e output: PE-transpose 128x128 blocks into one PSUM
                    # bank, bounce through SBUF (DMA can't read PSUM)
                    tp = tpool.tile([P, D], f32, tag="tp")
                    for eh in range(2):
                        esl = slice(eh * P, (eh + 1) * P)
                        nc.tensor.transpose(
                            out=tp[:, esl], in_=xef[:, esl], identity=ident[:]
                        )
                    xeT = gpool.tile([P, D], f32, tag="xeT")
                    nc.scalar.copy(out=xeT[:], in_=tp[:])
                    dout = xe_d[b].rearrange("(two e) hw -> e two hw", two=2)
                    nc.sync.dma_start(out=dout[:, :, msl], in_=xeT[:])

            nc.sync.dma_start(out=acc_d[:, 0:32], in_=acc_cross[:])
            nc.sync.dma_start(out=acc_d[:, 32:64], in_=acc_sxe[:])
            nc.sync.dma_start(out=acc_d[:, 64:72], in_=acc_sxx[:])

    nc.compile()
    return nc


def _get_program():
    if "nc" not in _CACHE:
        _CACHE["nc"] = _build_program()
    return _CACHE["nc"]


def _make_in_maps(x, lut):
    lutT = np.ascontiguousarray(lut.T)
    csq = (lut.astype(np.float64) ** 2).sum(axis=1)
    bias_rep = np.broadcast_to((-0.5 * csq).astype(np.float32).reshape(1, K), (128, K)).copy()
    in_maps = []
    for i in range(NCORES):
        in_maps.append({
            "x_sh": np.ascontiguousarray(
                x[i * BPC : (i + 1) * BPC].reshape(BPC, D, HW)),
            "lut": lut,
            "lutT": lutT,
            "bias_rep": bias_rep,
        })
    return in_maps


def kernel(x: np.ndarray, lookup_table: np.ndarray):
    x = np.asarray(x, dtype=np.float32)
    lut = np.ascontiguousarray(
        np.asarray(lookup_table, dtype=np.float32).reshape(K, D))

    from concourse.bass_utils import run_bass_kernel_spmd

    nc = _get_program()
    in_maps = _make_in_maps(x, lut)
    try:
        res = run_bass_kernel_spmd(nc, in_maps, list(range(NCORES))).results
    except Exception:
        # a previously wedged device recovers on the next attempt
        res = run_bass_kernel_spmd(nc, in_maps, list(range(NCORES))).results

    x_e = np.concatenate(
        [res[i]["xe_sh"].reshape(BPC, D, H, W) for i in range(NCORES)], axis=0)
    q_x = np.concatenate(
        [res[i]["q_sh"].astype(np.int32).reshape(BPC, H, W)
         for i in range(NCORES)], axis=0)

    tot = 0.0
    for i in range(NCORES):
        acc = res[i]["acc"].astype(np.float64)
        cross = acc[:, 0:32].sum()
        sxe = acc[:, 32:64].sum()
        sxx = acc[:, 64:72].sum()
        tot += sxx + sxe - 2.0 * cross
    loss = np.float32(tot / (B * D * HW))

    return x_e, q_x, loss
